# revision 17
# baseline (speedup 1.0000x reference)
"""Trainium2 Bass kernel for nn_CrossAttention_5265629905601.

Reference computation (per batch b):
    q = query @ Wq.T + bq            [S, O]
    k = key   @ Wk.T + bk            [S, O]
    v = value @ Wv.T + bv            [S, O]
    scores = (q @ k.T) * O**-0.5     [S, S]
    probs  = softmax(scores, -1)
    out    = probs @ v               [S, O]

Sharding: data-parallel over batch — 16 batches / 8 cores = 2 per core.

Algebraic restructuring (cuts device MACs/batch from 13.96G to ~9.7G and
keeps every contraction on the narrow DKV=768 axis):
    scores  = Q (Wq^T Wk) K^T + u 1^T + 1 w^T + c
  with A = Wq^T Wk [DQ,DKV] and w = K (Wk^T bq).  The u/c terms are
  row-constant so they cancel in the row softmax; w is added via the
  per-partition bias input of the Exp activation (scores are computed
  transposed: sT[t, s]).  On the output side,
    out = probs v = (probs V) Wv^T + bv
  because rows of probs sum to one.  The softmax denominator is obtained by
  appending a ones-column to V: C_aug = e^T [V | 1] gives the column sums in
  C_aug[:, 768] in exactly the layout (per-partition scalar over s) needed
  for the reciprocal-normalize of C.  bv is added by the DVE during the
  final PSUM->SBUF copy (host passes it partition-broadcast).

  A, Wv^T and the w-bias are tiny batch-independent (resp. O(S DKV))
  host-side weight preps.  All matmuls run in bf16 (fp32 PSUM accumulation).

Engine layout per batch (PE is the roofline):
  - K^T and Q^T come straight from HBM via XBAR DMA-transpose (2-byte
    dtype), so the only PE transposes left are the 6-per-s-block C
    transposes.  Input DMAs issue on the ACT sequencer, output stores on
    SP, so next-batch prefetch never queues behind current-batch stores.
  - Per 512-wide q-tile: B1T = A^T QT, sT = KT^T B1T,
    e = exp(scale*sT + w-bias) [ACT], C_aug = e^T [V|1],
    normalize by 1/colsum [DVE], PE-transpose C, out = C^T Wv^T (+bv, DVE),
    DMA out.
"""

import numpy as np
from contextlib import ExitStack

import concourse.bacc as bacc_mod
import concourse.tile as tile
import concourse.mybir as mybir
from concourse.bass_utils import run_bass_kernel_spmd

F32 = mybir.dt.float32
BF = mybir.dt.bfloat16
AF = mybir.ActivationFunctionType

P = 128
N_CORES = 8
B_TOTAL, S, DQ, DKV, O = 16, 2048, 1024, 768, 1024
B_PER = B_TOTAL // N_CORES          # batches per core
SCALE = float(O) ** -0.5            # 1/32

S_TILES = S // 512                  # 4  (512-wide q tiles)
K_BLKS = S // P                     # 16 (128-row key blocks)
DQC = DQ // P                       # 8  (query-feature 128-chunks)
DKC = DKV // P                      # 6  (kv-feature 128-chunks)


def build_nc(n_reps: int = 1):
    """Build + compile the per-core Bass program.  n_reps>1 wraps the whole
    body in a runtime loop (used only for hardware timing)."""
    nc = bacc_mod.Bacc("TRN2", target_bir_lowering=False, debug=False,
                       num_devices=N_CORES)

    query = nc.dram_tensor("query", [B_PER, S, DQ], BF, kind="ExternalInput")
    key = nc.dram_tensor("key", [B_PER, S, DKV], BF, kind="ExternalInput")
    value = nc.dram_tensor("value", [B_PER, S, DKV], BF, kind="ExternalInput")
    a_pp = nc.dram_tensor("a_pp", [P, DQC, DKV], BF, kind="ExternalInput")
    wvt_pp = nc.dram_tensor("wvt_pp", [P, DKC, O], BF, kind="ExternalInput")
    bv_bc = nc.dram_tensor("bv_bc", [P, O], F32, kind="ExternalInput")
    w_pp = nc.dram_tensor("w_pp", [B_PER, P, K_BLKS], F32, kind="ExternalInput")
    ident_in = nc.dram_tensor("ident_in", [P, P], BF, kind="ExternalInput")
    out = nc.dram_tensor("out", [B_PER, S, O], F32, kind="ExternalOutput")

    with tile.TileContext(nc) as tc, ExitStack() as top:
        singles = top.enter_context(tc.tile_pool(name="singles", bufs=1))
        a_sb = singles.tile([P, DQC, DKV], BF)
        nc.scalar.dma_start(a_sb, a_pp[:])
        ident = singles.tile([P, P], BF)
        nc.scalar.dma_start(ident, ident_in[:])
        wvt_sb = singles.tile([P, DKC, O], BF)
        nc.scalar.dma_start(wvt_sb, wvt_pp[:])
        bv_sb = singles.tile([P, O], F32)
        nc.scalar.dma_start(bv_sb, bv_bc[:])

        # PSUM: 1 transpose bank + 7 matmul banks = 8.
        psT = top.enter_context(tc.tile_pool(name="psT", bufs=1, space="PSUM"))
        psMM = top.enter_context(tc.tile_pool(name="psMM", bufs=7, space="PSUM"))

        # SBUF pools (top-level so consecutive batches double-buffer).
        resid = top.enter_context(tc.tile_pool(name="resid", bufs=2))
        qtp = top.enter_context(tc.tile_pool(name="qtp", bufs=2))
        b1p = top.enter_context(tc.tile_pool(name="b1p", bufs=2))
        ep = top.enter_context(tc.tile_pool(name="ep", bufs=18))
        cbp = top.enter_context(tc.tile_pool(name="cbp", bufs=3))
        ctp = top.enter_context(tc.tile_pool(name="ctp", bufs=3))
        osp = top.enter_context(tc.tile_pool(name="osp", bufs=3))
        rcp = top.enter_context(tc.tile_pool(name="rcp", bufs=4))

        def transpose_group(src_fn, dst, n_chunks):
            """PE-transpose n_chunks 128x128 bf16 blocks; batch 4 per PSUM
            bank and copy out with one wide DVE copy per bank.
            src_fn(dc) -> [128,128] bf16 AP; dst: bf16 AP [128, n_chunks, 128].
            """
            for g0 in range(0, n_chunks, 4):
                gw = min(4, n_chunks - g0)
                tps = psT.tile([P, 512], BF, tag="tps")
                for j in range(gw):
                    nc.tensor.transpose(tps[:, j * P:(j + 1) * P],
                                        src_fn(g0 + j), ident)
                nc.vector.tensor_copy(
                    dst[:, g0:g0 + gw, :],
                    tps[:, :gw * P].rearrange("p (d c) -> p d c", d=gw))

        def emit_batch(b):
            KT = resid.tile([P, DKC, S], BF, tag="KT")
            vsb = resid.tile([P, K_BLKS, DKV + 1], BF, tag="vsb")
            wsb = resid.tile([P, K_BLKS], F32, tag="wsb")
            nc.scalar.dma_start(wsb, w_pp[b])

            # Q^T / K^T straight from HBM via XBAR transpose (per 128-col
            # chunk).  ALL XBAR-transpose DMAs must share one engine queue
            # (SP): concurrent transposes from two HWDGE queues corrupt each
            # other (verified on HW); normal DMAs on the other queue are safe.
            def issue_qT(qt):
                qT = qtp.tile([P, DQC, 512], BF, tag="qT")
                for dqc in range(DQC):
                    nc.sync.dma_start(
                        qT[:, dqc],
                        query[b, qt * 512:(qt + 1) * 512,
                              dqc * P:(dqc + 1) * P],
                        transpose=True)
                return qT

            # qT(0) first: B1T only needs a_sb+qT, so PE can start earliest.
            qT_next = issue_qT(0)
            for dc in range(DKC):
                nc.sync.dma_start(KT[:, dc], key[b][:, dc * P:(dc + 1) * P],
                                  transpose=True)

            for qt in range(S_TILES):
                qT = qT_next
                if qt + 1 < S_TILES:
                    qT_next = issue_qT(qt + 1)

                # B1T[d, s] = A^T QT  (accumulate over the 8 dq chunks)
                b1 = b1p.tile([P, DKC, 512], BF, tag="b1")
                for dc in range(DKC):
                    ps = psMM.tile([P, 512], F32, tag="mm")
                    for dqc in range(DQC):
                        nc.tensor.matmul(
                            ps, a_sb[:, dqc, dc * P:(dc + 1) * P],
                            qT[:, dqc, :],
                            start=(dqc == 0), stop=(dqc == DQC - 1))
                    nc.vector.tensor_copy(b1[:, dc, :], ps)

                if qt == 0:
                    # V arrives behind KT in the DMA queues; C(0) only needs
                    # it after scores+exp, so issue it after B1T.
                    nc.scalar.dma_start(
                        vsb[:, :, 0:DKV],
                        value[b].rearrange("(tb p) d -> p tb d", p=P))
                    nc.vector.memset(vsb[:, :, DKV:DKV + 1], 1.0)

                # tail out-stages of the previous q-tile, hidden under B1T
                for f in pending:
                    f()
                pending.clear()

                # scores (transposed) + exp with w bias
                e_tiles = []
                for tb in range(K_BLKS):
                    s_ps = psMM.tile([P, 512], F32, tag="mm")
                    for dc in range(DKC):
                        nc.tensor.matmul(
                            s_ps, KT[:, dc, tb * P:(tb + 1) * P],
                            b1[:, dc, :],
                            start=(dc == 0), stop=(dc == DKC - 1))
                    e_t = ep.tile([P, 512], BF, tag="E")
                    nc.scalar.activation(e_t, s_ps, AF.Exp, scale=SCALE,
                                         bias=wsb[:, tb:tb + 1])
                    e_tiles.append(e_t)

                # C_aug = e^T [V|1]; normalize; transpose; out = C^T Wv^T + bv
                # Pipelined so the DVE normalize/copy for C(sc) always hides
                # under a full C-chain of PE work:
                #   C0 C1 ct0 C2 out0 ct1 C3 out1 ct2 out2 ct3 out3
                cs = [None] * 4     # (clo, chi)
                cts = [None] * 4    # ct tiles

                def emit_C(sc):
                    clo = psMM.tile([P, 512], F32, tag="mm")
                    chi = psMM.tile([P, 512], F32, tag="mm")
                    for tb in range(K_BLKS):
                        st_ap = e_tiles[tb][:, sc * P:(sc + 1) * P]
                        nc.tensor.matmul(clo, st_ap, vsb[:, tb, 0:512],
                                         start=(tb == 0), stop=(tb == K_BLKS - 1))
                        nc.tensor.matmul(chi[:, 0:257], st_ap,
                                         vsb[:, tb, 512:DKV + 1],
                                         start=(tb == 0), stop=(tb == K_BLKS - 1))
                    cs[sc] = (clo, chi)

                def emit_ct(sc):
                    clo, chi = cs[sc]
                    rcs = rcp.tile([P, 1], F32, tag="rcs")
                    nc.vector.reciprocal(rcs, chi[:, 256:257])
                    cbf = cbp.tile([P, DKV], BF, tag="cbf")
                    nc.vector.tensor_scalar_mul(cbf[:, 0:512], clo, rcs)
                    nc.vector.tensor_scalar_mul(cbf[:, 512:768],
                                                chi[:, 0:256], rcs)
                    ct = ctp.tile([P, DKC, P], BF, tag="ct")
                    transpose_group(
                        lambda dc: cbf[:, dc * P:(dc + 1) * P], ct, DKC)
                    cts[sc] = ct

                def emit_out(sc, qt=qt, cts=cts):
                    # bind qt/cts by value: emit_out(2)/(3) run deferred, after
                    # the loop variables have been rebound for the next q-tile
                    ct = cts[sc]
                    for oh in range(2):
                        o_ps = psMM.tile([P, 512], F32, tag="mm")
                        for dc in range(DKC):
                            nc.tensor.matmul(
                                o_ps, ct[:, dc, :],
                                wvt_sb[:, dc, oh * 512:(oh + 1) * 512],
                                start=(dc == 0), stop=(dc == DKC - 1))
                        o_sb = osp.tile([P, 512], F32, tag="osb")
                        nc.vector.tensor_add(
                            o_sb, o_ps, bv_sb[:, oh * 512:(oh + 1) * 512])
                        nc.scalar.dma_start(
                            out[b, qt * 512 + sc * P: qt * 512 + (sc + 1) * P,
                                oh * 512:(oh + 1) * 512], o_sb)

                emit_C(0); emit_C(1); emit_ct(0); emit_C(2); emit_out(0)
                emit_ct(1); emit_C(3); emit_out(1); emit_ct(2)
                emit_ct(3)
                # Defer only SBUF-consuming stages: deferring a PSUM reader
                # (e.g. ct(3)'s normalize) past the next tile's PSUM
                # allocations breaks the psMM pool's FIFO free order and
                # corrupts live banks (observed on HW).
                pending.append(lambda o=emit_out: (o(2), o(3)))

        pending = []

        def body():
            for b in range(B_PER):
                emit_batch(b)
            for f in pending:
                f()
            pending.clear()

        if n_reps > 1:
            with tc.For_i(0, n_reps) as _i:
                body()
        else:
            body()

    nc.compile()
    return nc


_nc_cache = {}


def _get_nc(n_reps: int = 1):
    if n_reps not in _nc_cache:
        _nc_cache[n_reps] = build_nc(n_reps)
    return _nc_cache[n_reps]


def make_in_maps(query, key, value, Wq, bq, Wk, bk, Wv, bv):
    """Host-side prep: shard activations over batch; fold the weights."""
    BFn = mybir.dt.np(BF)
    query = np.asarray(query, dtype=np.float32)
    key = np.asarray(key, dtype=np.float32)
    value = np.asarray(value, dtype=np.float32)
    Wq = np.asarray(Wq, np.float32)
    Wk = np.asarray(Wk, np.float32)
    Wv = np.asarray(Wv, np.float32)
    bq = np.asarray(bq, np.float32)
    bv = np.asarray(bv, np.float32)

    A = Wq.T @ Wk                               # [DQ, DKV]
    g = Wk.T @ bq                               # [DKV]
    w = SCALE * (key @ g)                       # [B, S]
    w_pp = np.ascontiguousarray(
        w.reshape(B_TOTAL, K_BLKS, P).transpose(0, 2, 1))   # [B, P, K_BLKS]

    shared = {
        "a_pp": np.ascontiguousarray(
            A.reshape(DQC, P, DKV).transpose(1, 0, 2).astype(BFn)),
        "wvt_pp": np.ascontiguousarray(
            Wv.T.reshape(DKC, P, O).transpose(1, 0, 2).astype(BFn)),
        "bv_bc": np.ascontiguousarray(
            np.broadcast_to(bv.reshape(1, O), (P, O)).astype(np.float32)),
        "ident_in": np.eye(P, dtype=BFn),
    }
    q_bf = query.astype(BFn)
    k_bf = key.astype(BFn)
    v_bf = value.astype(BFn)
    in_maps = []
    for c in range(N_CORES):
        sl = slice(c * B_PER, (c + 1) * B_PER)
        in_maps.append({
            "query": q_bf[sl], "key": k_bf[sl], "value": v_bf[sl],
            "w_pp": w_pp[sl], **shared,
        })
    return in_maps


def kernel(query, key, value, Wq, bq, Wk, bk, Wv, bv):
    in_maps = make_in_maps(query, key, value, Wq, bq, Wk, bk, Wv, bv)
    nc = _get_nc(1)
    res = run_bass_kernel_spmd(nc, in_maps, core_ids=list(range(N_CORES)))
    return np.concatenate([r["out"] for r in res.results], axis=0)
